# revision 24
# baseline (speedup 1.0000x reference)
"""Trainium2 Bass kernel for nn_DiagMatrixConstructionBlock.

Sharding: one graph per NeuronCore (B=8 graphs, 8 cores). Each core gets its
graph's nodes zero-padded to 128 rows; the whole pipeline is linear/bilinear
with no bias, so padded rows yield exactly-zero output blocks, matching
to_dense_batch semantics.
"""
import math
import os
import sys
from functools import lru_cache

import numpy as np

F = 48
K = 16
P = 2
B = 8
D = 192          # 4*F
NMAX = 128
NFEAT = 432      # 9*F

PATHS = [(0, 0, 0), (0, 1, 1), (0, 2, 2), (1, 0, 1), (1, 1, 0), (1, 1, 2),
         (1, 2, 1), (2, 0, 2), (2, 1, 1), (2, 2, 0), (2, 2, 2)]
LOFF = {0: 0, 1: 48, 2: 192}          # column offset of l-block in node_feats
NP_L = {0: 3, 1: 4, 2: 4}             # number of paths feeding each l3


def _su2_cg(j1, j2, j3, m1, m2, m3):
    if m1 + m2 != m3:
        return 0.0
    f = math.factorial
    pre = ((2 * j3 + 1) * f(j3 + j1 - j2) * f(j3 - j1 + j2) * f(j1 + j2 - j3) / f(j1 + j2 + j3 + 1)) ** 0.5
    pre *= (f(j3 + m3) * f(j3 - m3) * f(j1 - m1) * f(j1 + m1) * f(j2 - m2) * f(j2 + m2)) ** 0.5
    s = 0.0
    for v in range(0, j1 + j2 - j3 + 1):
        args = [v, j1 + j2 - j3 - v, j1 - m1 - v, j2 + m2 - v, j3 - j2 + m1 + v, j3 - j1 - m2 + v]
        if min(args) < 0:
            continue
        den = 1
        for a in args:
            den *= f(a)
        s += (-1) ** v / den
    return pre * s


def _u_c2r(l):
    U = np.zeros((2 * l + 1, 2 * l + 1), dtype=complex)
    s2 = 2 ** -0.5
    for m in range(-l, l + 1):
        a = l + m
        if m > 0:
            U[a, l + m] = (-1) ** m * s2
            U[a, l - m] = s2
        elif m == 0:
            U[a, l] = 1.0
        else:
            U[a, l + m] = 1j * s2
            U[a, l - m] = -1j * (-1) ** m * s2
    return U


def _real_cg(l1, l2, l3):
    Cc = np.zeros((2 * l1 + 1, 2 * l2 + 1, 2 * l3 + 1), dtype=complex)
    for a, m1 in enumerate(range(-l1, l1 + 1)):
        for b, m2 in enumerate(range(-l2, l2 + 1)):
            for c, m3 in enumerate(range(-l3, l3 + 1)):
                Cc[a, b, c] = _su2_cg(l1, l2, l3, m1, m2, m3)
    U1, U2, U3 = _u_c2r(l1), _u_c2r(l2), _u_c2r(l3)
    C = np.einsum('ap,bq,cr,pqr->abc', np.conj(U1), np.conj(U2), U3, Cc)
    C = C.imag if np.abs(C.imag).max() > np.abs(C.real).max() else C.real
    nrm = np.linalg.norm(C)
    return np.asarray(C / max(nrm, 1e-12), dtype=np.float32)


CG = {lls: _real_cg(*lls) for lls in set(PATHS) | {(1, 1, 2)}}


def _triples():
    """All nonzero CG entries: (path, l1, l2, l3, m, k, c, val)."""
    out = []
    for p, (l1, l2, l3) in enumerate(PATHS):
        C = CG[(l1, l2, l3)]
        for m in range(2 * l1 + 1):
            for k in range(2 * l2 + 1):
                for c in range(2 * l3 + 1):
                    v = float(np.float32(C[m, k, c]))
                    if abs(v) > 1e-8:
                        out.append((p, l1, l2, l3, m, k, c, v))
    return out


TRIPLES = _triples()

# variant table: (path, val) -> (w2s column offset, width)
def _variants():
    offs = {}
    widths = {}
    tot = 0
    for (p, l1, l2, l3, m, k, c, v) in TRIPLES:
        key = (p, v)
        if key not in offs:
            w = 96 if l3 == 0 else 48
            offs[key] = tot
            widths[key] = w
            tot += w
    return offs, widths, tot


W2S_OFF, W2S_W, W2S_TOT = _variants()

_CACHE = {}


def _build_nc(dense: bool, prec: str = "f32r", nw: int = NMAX):
    sys.path.insert(0, '/opt/trn_rl_repo')
    import concourse.bass as bass
    import concourse.mybir as mybir
    import concourse.tile as tile
    from concourse import bacc

    fp32 = mybir.dt.float32
    AOP = mybir.AluOpType
    # adt: linear-chain dtype (wls/tpt/XT/Y); pdt: products + w2s dtype
    adt = {"f32r": mybir.dt.float32r, "mixed": mybir.dt.float16,
           "bf16": mybir.dt.bfloat16, "fp32": mybir.dt.float32}[prec]
    pdt = {"f32r": mybir.dt.float32r, "mixed": mybir.dt.float16,
           "bf16": mybir.dt.bfloat16, "fp32": mybir.dt.float32}[prec]
    nc = bacc.Bacc("TRN2", target_bir_lowering=False)
    # x arrives host-transposed+planar: [48 u, (l,i)-planes * 128 nodes]
    x = nc.dram_tensor("x", [F, 9 * 128], adt, kind="ExternalInput")
    wls = nc.dram_tensor("wls", [F, 3 * F], adt, kind="ExternalInput")
    tpt = nc.dram_tensor("tpt", [F, 11 * F], adt, kind="ExternalInput")
    w2s = nc.dram_tensor("w2s", [F, W2S_TOT], pdt, kind="ExternalInput")
    out = nc.dram_tensor("out", [NMAX, P, D, D], fp32, kind="ExternalOutput")
    out_flat = out.rearrange("n p a b -> n p (a b)")
    dbg = os.environ.get("BASS_KERNEL_DEBUG", "0") == "1"
    if dbg:
        dbg_y = {l: nc.dram_tensor(f"dbg_y{l}", [F, (2 * l + 1) * 128], fp32,
                                   kind="ExternalOutput") for l in (0, 1, 2)}
        dbg_g = {l: nc.dram_tensor(f"dbg_g{l}", [NMAX, (96, 144, 240)[l]], fp32,
                                   kind="ExternalOutput") for l in (0, 1, 2)}
        dbg_prod = nc.dram_tensor("dbg_prod", [F, 25 * 128], fp32,
                                  kind="ExternalOutput")

    C112 = CG[(1, 1, 2)]
    s3 = float(np.float32(3.0 ** -0.5))

    def apv(base, extra_off, dims):
        """Custom strided free-dim view: keep partition dim, replace free dims."""
        ap = base.copy()
        newap = [list(ap.ap[0])] + [list(dd) for dd in dims]
        return ap.__replace__(offset=ap.offset + extra_off, ap=newap)

    with tile.TileContext(nc) as tc:
        with tc.tile_pool(name="const", bufs=1) as cpool, \
             tc.tile_pool(name="work", bufs=1) as wpool, \
             tc.tile_pool(name="xtp", bufs=2) as xtpool, \
             tc.tile_pool(name="pp", bufs=3) as ppool, \
             tc.tile_pool(name="psy", bufs=1, space="PSUM") as psy, \
             tc.tile_pool(name="psa", bufs=2, space="PSUM") as psa, \
             tc.tile_pool(name="psg", bufs=1, space="PSUM") as psg:

            xt = wpool.tile([F, 9 * 128], adt, tag="x")
            nc.sync.dma_start(out=xt[:], in_=x[:])
            wls_sb = cpool.tile([F, 3 * F], adt, tag="wls")
            nc.sync.dma_start(out=wls_sb[:], in_=wls[:])
            tpt_sb = cpool.tile([F, 11 * F], adt, tag="tpt")
            nc.sync.dma_start(out=tpt_sb[:], in_=tpt[:])
            w2s_sb = cpool.tile([F, W2S_TOT], pdt, tag="w2s")
            nc.sync.dma_start(out=w2s_sb[:], in_=w2s[:])

            if dense:
                img = wpool.tile([NMAX, D * D], fp32, tag="img")
                nc.gpsimd.memset(img[:], 0.0)

            # ---- PE warm-up: ~4us of dummy matmuls on the first-loaded
            # weight tile so HAM reaches K=8/8 before the real work ----
            wsrc = cpool.tile([F, 512], mybir.dt.bfloat16, tag="wsrc")
            nc.vector.memset(wsrc[:], 0.0)
            wup = psy.tile([F, 512], fp32, tag="warm", name="warm")
            for _ in range(9):
                nc.tensor.matmul(wup[:], wsrc[:, 0:48], wsrc[:],
                                 start=True, stop=True)

            # ---- y ----
            Y = {}
            PLOFF = {0: 0, 1: 128, 2: 4 * 128}  # plane offsets in xt columns
            for l in (0, 1, 2):
                ni = 2 * l + 1
                Y[l] = wpool.tile([F, ni * 128], adt, tag=f"y{l}", name=f"y{l}")
                for c0 in range(0, ni * 128, 512):
                    c1 = min(c0 + 512, ni * 128)
                    pyy = psy.tile([F, 512], fp32, tag="yps")
                    nc.tensor.matmul(pyy[:, 0:c1 - c0],
                                     wls_sb[:, l * F:(l + 1) * F],
                                     xt[:, PLOFF[l] + c0:PLOFF[l] + c1],
                                     start=True, stop=True)
                    nc.vector.tensor_copy(Y[l][:, c0:c1], pyy[:, 0:c1 - c0])

            if dbg:
                for l in (0, 1, 2):
                    nc.sync.dma_start(out=dbg_y[l][:], in_=Y[l][:])

            # ---- g accumulators in PSUM ----
            G = {0: psg.tile([NMAX, 96], fp32, tag="g0", name="g0"),
                 1: psg.tile([NMAX, 144], fp32, tag="g1", name="g1"),
                 2: psg.tile([NMAX, 240], fp32, tag="g2", name="g2")}

            # start/stop at PSUM-bank granularity: one accumulation group
            # per G tile (start marks the whole 2KB zero-region pending).
            emit_order = [t for p in ([q for q in range(11) if PATHS[q][2] < 2]
                                      + [q for q in range(11) if PATHS[q][2] == 2])
                          for t in TRIPLES if t[0] == p]
            bank_trips = {}
            for t in emit_order:
                bank_trips.setdefault(t[3], []).append(t)
            first_of_bank = {b: ts[0] for b, ts in bank_trips.items()}
            last_of_bank = {b: ts[-1] for b, ts in bank_trips.items()}

            # ---- per-path: a matmul, products, triple matmuls ----
            # l3=2 paths last so the i=0 output rows (needing only g0/g1)
            # can be assembled + scattered while l3=2 still computes
            path_order = [p for p in range(11) if PATHS[p][2] < 2] + \
                         [p for p in range(11) if PATHS[p][2] == 2]
            for p in path_order:
                (l1, l2, l3) = PATHS[p]
                m1n = 2 * l1 + 1
                k2n = 2 * l2 + 1
                # a matmuls, in k-groups that fit one PSUM bank (<=4 k-planes)
                kgs = [(0, min(4, k2n))] + ([(4, k2n)] if k2n > 4 else [])
                prod = ppool.tile([F, m1n, k2n, 128], pdt, tag="prod")
                a16 = xtpool.tile([F, 5 * 128], adt, tag="a16", name="a16")
                for (k0, k1) in kgs:
                    kw = k1 - k0
                    pa = psa.tile([F, 4 * 128], fp32, tag="aps")
                    nc.tensor.matmul(pa[:, 0:kw * 128],
                                     tpt_sb[:, p * F:(p + 1) * F],
                                     Y[l2][:, k0 * 128:k1 * 128],
                                     start=True, stop=True)
                    nc.scalar.copy(a16[:, k0 * 128:k1 * 128], pa[:, 0:kw * 128])
                # products: P[m, k, n] = y1[m, n] * a[k, n]
                y1v = Y[l1][:].rearrange("p (m n) -> p m n", m=m1n)
                y1b = y1v[:, :, None, :].to_broadcast((F, m1n, k2n, 128))
                av = a16[:, 0:k2n * 128].rearrange("p (k n) -> p k n", k=k2n)
                ab = av[:, None, :, :].to_broadcast((F, m1n, k2n, 128))
                nc.vector.tensor_tensor(out=prod[:], in0=y1b, in1=ab,
                                        op=AOP.mult)
                if dbg and p == 10:
                    nc.sync.dma_start(out=dbg_prod[:],
                                      in_=prod[:].rearrange("p a b c -> p (a b c)"))
                # triple matmuls for this path
                for t in TRIPLES:
                    tp, _, _, tl3, m, k, c, v = t
                    if tp != p:
                        continue
                    off = W2S_OFF[(p, v)]
                    w = W2S_W[(p, v)]
                    if tl3 == 0:
                        gdst = G[0][:, 0:96]
                    else:
                        gdst = G[tl3][:, c * F:(c + 1) * F]
                    nc.tensor.matmul(gdst,
                                     prod[:, m, k, :],
                                     w2s_sb[:, off:off + w],
                                     start=(first_of_bank[tl3] is t),
                                     stop=(last_of_bank[tl3] is t),
                                     skip_group_check=True)

            if dbg:
                for l in (0, 1, 2):
                    gs = wpool.tile([NMAX, (96, 144, 240)[l]], fp32, tag=f"dbggs{l}", name=f"dbggs{l}")
                    nc.vector.tensor_copy(gs[:], G[l][:])
                    nc.sync.dma_start(out=dbg_g[l][:], in_=gs[:])

            # ---- M assembly ----
            C112v = CG[(1, 1, 2)]
            s3c = float(np.float32(3.0 ** -0.5))
            a0 = G[0][:, 0:48]
            g1v = apv(G[1][:], 0, [[1, 48], [48, 3]])
            a1b = apv(G[0][:], 48, [[1, 48], [0, 3]])
            a1b1 = apv(G[0][:], 48, [[1, 48]])
            d2b2 = apv(G[2][:], 2 * 48, [[1, 48], [0, 2]])
            d2b1 = apv(G[2][:], 2 * 48, [[1, 48]])
            d4b1 = apv(G[2][:], 4 * 48, [[1, 48]])

            def assemble(dst, fs, istr, jstr, i0dst=None, i0fs=0, i0js=0):
                """Write M values. If i0dst given, row i=0 goes there
                (i0fs/i0js strides), rows i>=1 go to dst with (i-1) indexing
                via istr."""
                dfull = dst[:]

                def dv(off, dims):
                    return apv(dfull, off, dims)

                if i0dst is None:
                    t, tfs, tjs, ioff = dst, fs, jstr, 0
                else:
                    t, tfs, tjs, ioff = i0dst, i0fs, i0js, 0
                tf = t[:]
                nc.vector.tensor_copy(apv(tf, ioff, [[tfs, 48]]), a0)
                nc.vector.tensor_copy(apv(tf, ioff + tjs, [[tfs, 48], [tjs, 3]]),
                                      g1v)
                # base offset of row i (i>=1) inside dst
                def ro(i, j):
                    ii = i - 1 if i0dst is not None else i
                    return ii * istr + j * jstr
                # first column (i,0) = v
                nc.vector.tensor_copy(dv(ro(1, 0), [[fs, 48], [istr, 3]]), g1v)
                # diag a1/sqrt3
                dstep = ro(2, 2) - ro(1, 1)
                nc.vector.tensor_scalar_mul(
                    dv(ro(1, 1), [[fs, 48], [dstep, 3]]), a1b, s3c)
                nc.vector.scalar_tensor_tensor(
                    out=dv(ro(1, 1), [[fs, 48], [2 * dstep, 2]]), in0=d2b2,
                    scalar=float(C112v[0, 0, 2]),
                    in1=dv(ro(1, 1), [[fs, 48], [2 * dstep, 2]]),
                    op0=AOP.mult, op1=AOP.add)
                nc.vector.scalar_tensor_tensor(
                    out=dv(ro(2, 2), [[fs, 48]]), in0=d2b1,
                    scalar=float(C112v[1, 1, 2]),
                    in1=dv(ro(2, 2), [[fs, 48]]), op0=AOP.mult, op1=AOP.add)
                nc.vector.scalar_tensor_tensor(
                    out=dv(ro(1, 1), [[fs, 48]]), in0=d4b1,
                    scalar=float(C112v[0, 0, 4]),
                    in1=dv(ro(1, 1), [[fs, 48]]), op0=AOP.mult, op1=AOP.add)
                nc.vector.scalar_tensor_tensor(
                    out=dv(ro(3, 3), [[fs, 48]]), in0=d4b1,
                    scalar=float(C112v[2, 2, 4]),
                    in1=dv(ro(3, 3), [[fs, 48]]), op0=AOP.mult, op1=AOP.add)
                for (dm, r, cc, coef) in ((1, 1, 2, float(C112v[0, 1, 1])),
                                          (0, 1, 3, float(C112v[0, 2, 0])),
                                          (3, 2, 3, float(C112v[1, 2, 3]))):
                    o1, o2 = ro(r, cc), ro(cc, r)
                    dmb = apv(G[2][:], dm * 48, [[1, 48], [0, 2]])
                    nc.vector.tensor_scalar_mul(
                        dv(min(o1, o2), [[fs, 48], [abs(o2 - o1), 2]]),
                        dmb, coef)

            if dense:
                img_t = img
                assemble(img_t, 772, 192, 1)
                for pp_ in range(P):
                    nc.sync.dma_start(out=out_flat[0:nw, pp_], in_=img[0:nw, :])
            else:
                m_i0 = wpool.tile([NMAX, 4 * 48], fp32, tag="mi0", name="mi0")
                m_rest = wpool.tile([NMAX, 12 * 48], fp32, tag="mrest",
                                    name="mrest")
                assemble(m_rest, 12, 4, 1, i0dst=m_i0, i0fs=4, i0js=1)
                mi0v = m_i0[0:nw, :].rearrange("n (f j) -> n f j", j=4)
                mrv = m_rest[0:nw, :].rearrange("n (f i j) -> n f i j", i=3, j=4)
                def dap_for(pp_, i):
                    base = out_flat[:, pp_]
                    nap = [list(base.ap[0]), [772, 48], [1, 4]]
                    nap[0][1] = nw
                    return base.__replace__(
                        offset=base.offset + 192 * i, ap=nap)
                for pp_ in range(P):
                    nc.gpsimd.dma_start(out=dap_for(pp_, 0), in_=mi0v)
                for pp_ in range(P):
                    for i in range(1, 4):
                        nc.gpsimd.dma_start(out=dap_for(pp_, i),
                                            in_=mrv[:, :, i - 1, :])
    nc.finalize()
    return nc


def _get_nc(dense, prec, nw=NMAX):
    key = (dense, prec, nw)
    if key not in _CACHE:
        _CACHE[key] = _build_nc(dense, prec, nw)
    return _CACHE[key]


def _host_weights(W_lin0, W_lin1, W_lin2, tp_w, W_out0, W_out1, W_out2,
                  Wt0, Wt1, Wt2, prec="f32r"):
    sF = np.float32(F) ** -0.5
    sK = np.float32(K) ** -0.5
    wls = np.concatenate([W_lin0, W_lin1, W_lin2], axis=1) * sF      # [48,144]
    tpt = np.concatenate([tp_w[p].T for p in range(11)], axis=1) * sF
    Wt = {0: Wt0, 1: Wt1, 2: Wt2}
    Wo = {0: W_out0, 1: W_out1, 2: W_out2}
    # W2_l[u', f, o] with h-norm and K-norm folded
    W2 = {}
    for l in (0, 1, 2):
        nrm = np.float32(NP_L[l] * F) ** -0.5
        W2[l] = np.einsum('ukf,ko->ufo',
                          Wo[l].reshape(-1, K, F), Wt[l]) * (nrm * sK)
    w2s = np.zeros((F, W2S_TOT), np.float32)
    for (p, v), off in W2S_OFF.items():
        l3 = PATHS[p][2]
        block = W2[l3][p_slice_start(p) * F:(p_slice_start(p) + 1) * F]
        if l3 == 0:
            # columns o*48+f
            w2s[:, off:off + 96] = (v * block.transpose(0, 2, 1)
                                    .reshape(F, 96))
        else:
            w2s[:, off:off + 48] = v * block[:, :, 0].reshape(F, F).T.T
    import ml_dtypes
    adt = {"f32r": np.float32, "mixed": np.float16,
           "bf16": ml_dtypes.bfloat16, "fp32": np.float32}[prec]
    pdt = {"f32r": np.float32, "mixed": np.float16,
           "bf16": ml_dtypes.bfloat16, "fp32": np.float32}[prec]
    return (np.ascontiguousarray(wls.astype(adt)),
            np.ascontiguousarray(tpt.astype(adt)),
            np.ascontiguousarray(w2s.astype(pdt)))


def p_slice_start(p):
    """index of path p within its l3 group (for W_out row blocks)."""
    l3 = PATHS[p][2]
    return sum(1 for q in range(p) if PATHS[q][2] == l3)


def _shard_nodes(node_feats, batch, prec="mixed"):
    import ml_dtypes
    adt = {"f32r": np.float32, "mixed": np.float16,
           "bf16": ml_dtypes.bfloat16, "fp32": np.float32}[prec]
    nf = np.asarray(node_feats, np.float32)
    bt = np.asarray(batch).astype(np.int64)
    xs = np.zeros((B, NMAX, NFEAT), np.float32)
    cnts = np.zeros(B, np.int32)
    for g in range(B):
        rows = nf[bt == g]
        cnt = min(len(rows), NMAX)
        xs[g, :cnt] = rows[:cnt]
        cnts[g] = cnt
    # transpose to [48 u, (l,i)-planes * 128] planar layout
    xtp = np.zeros((B, F, 9 * 128), np.float32)
    col = 0
    for l in (0, 1, 2):
        ni = 2 * l + 1
        for i in range(ni):
            blk = xs[:, :, LOFF[l] + i:LOFF[l] + ni * F:ni]   # [B, 128, 48]
            xtp[:, :, col * 128:(col + 1) * 128] = blk.transpose(0, 2, 1)
            col += 1
    return np.ascontiguousarray(xtp.astype(adt)), cnts


def _install_ntff_hook():
    """Provide the antenv.axon_hooks module the boot silently skipped, and
    register the ctypes-based NTFF profile hook so trace=True works."""
    try:
        sys.path.insert(0, '/root/.axon_site')
        import antenv
        shim = os.path.join(os.path.dirname(os.path.abspath(__file__)),
                            'work', 'shim_ax')
        if shim not in antenv.__path__ and os.path.isdir(shim):
            antenv.__path__.append(shim)
        import importlib
        try:
            ah = importlib.import_module('antenv.axon_hooks')
        except ImportError:
            return False
        if ah.get_axon_ntff_profile_hook() is None:
            from trn_agent_boot.trn_boot import _ntff_profile_via_ctypes
            ah.set_axon_ntff_profile_hook(
                _ntff_profile_via_ctypes('/opt/axon/libaxon_pjrt.so'))
        return True
    except Exception:
        return False


def _run(inputs, trace=False):
    dense = os.environ.get("BASS_KERNEL_DENSE", "0") == "1"
    prec = os.environ.get("BASS_KERNEL_PREC", "mixed")
    sys.path.insert(0, '/opt/trn_rl_repo')
    if trace:
        _install_ntff_hook()
    from concourse.bass_utils import run_bass_kernel_spmd
    wls, tpt, w2s = _host_weights(
        np.asarray(inputs['W_lin0'], np.float32),
        np.asarray(inputs['W_lin1'], np.float32),
        np.asarray(inputs['W_lin2'], np.float32),
        np.asarray(inputs['tp_w'], np.float32),
        np.asarray(inputs['W_out0'], np.float32),
        np.asarray(inputs['W_out1'], np.float32),
        np.asarray(inputs['W_out2'], np.float32),
        np.asarray(inputs['Wt0'], np.float32),
        np.asarray(inputs['Wt1'], np.float32),
        np.asarray(inputs['Wt2'], np.float32), prec=prec)
    xs, cnts = _shard_nodes(inputs['node_feats'], inputs['batch'], prec)
    # bake the max populated-slot count into the compiled output DMAs;
    # slots beyond it stay runtime-pre-zeroed
    nw = min(NMAX, int(-(-int(cnts.max()) // 8) * 8)) if cnts.max() > 0 else 8
    nc = _get_nc(dense, prec, nw)
    in_maps = [{"x": np.ascontiguousarray(xs[g]), "wls": wls, "tpt": tpt,
                "w2s": w2s} for g in range(B)]
    res = run_bass_kernel_spmd(nc, in_maps, core_ids=list(range(B)),
                               trace=trace)
    stacked = np.stack([np.asarray(res.results[g]["out"]) for g in range(B)])
    max_nodes = int(np.asarray(inputs['max_nodes']))
    if max_nodes == NMAX:
        full = stacked
    elif max_nodes < NMAX:
        full = stacked[:, :max_nodes]
    else:
        full = np.zeros((B, max_nodes, P, D, D), np.float32)
        full[:, :NMAX] = stacked
    return full, res


def kernel(**inputs):
    full, _ = _run(inputs, trace=False)
    return full


# revision 25
# speedup vs baseline: 1.7939x; 1.7939x over previous
"""Trainium2 Bass kernel for nn_DiagMatrixConstructionBlock.

Sharding: one graph per NeuronCore (B=8 graphs, 8 cores). Each core gets its
graph's nodes zero-padded to 128 rows; the whole pipeline is linear/bilinear
with no bias, so padded rows yield exactly-zero output blocks, matching
to_dense_batch semantics.
"""
import math
import os
import sys
from functools import lru_cache

import numpy as np

F = 48
K = 16
P = 2
B = 8
D = 192          # 4*F
NMAX = 128
NFEAT = 432      # 9*F

PATHS = [(0, 0, 0), (0, 1, 1), (0, 2, 2), (1, 0, 1), (1, 1, 0), (1, 1, 2),
         (1, 2, 1), (2, 0, 2), (2, 1, 1), (2, 2, 0), (2, 2, 2)]
LOFF = {0: 0, 1: 48, 2: 192}          # column offset of l-block in node_feats
NP_L = {0: 3, 1: 4, 2: 4}             # number of paths feeding each l3


def _su2_cg(j1, j2, j3, m1, m2, m3):
    if m1 + m2 != m3:
        return 0.0
    f = math.factorial
    pre = ((2 * j3 + 1) * f(j3 + j1 - j2) * f(j3 - j1 + j2) * f(j1 + j2 - j3) / f(j1 + j2 + j3 + 1)) ** 0.5
    pre *= (f(j3 + m3) * f(j3 - m3) * f(j1 - m1) * f(j1 + m1) * f(j2 - m2) * f(j2 + m2)) ** 0.5
    s = 0.0
    for v in range(0, j1 + j2 - j3 + 1):
        args = [v, j1 + j2 - j3 - v, j1 - m1 - v, j2 + m2 - v, j3 - j2 + m1 + v, j3 - j1 - m2 + v]
        if min(args) < 0:
            continue
        den = 1
        for a in args:
            den *= f(a)
        s += (-1) ** v / den
    return pre * s


def _u_c2r(l):
    U = np.zeros((2 * l + 1, 2 * l + 1), dtype=complex)
    s2 = 2 ** -0.5
    for m in range(-l, l + 1):
        a = l + m
        if m > 0:
            U[a, l + m] = (-1) ** m * s2
            U[a, l - m] = s2
        elif m == 0:
            U[a, l] = 1.0
        else:
            U[a, l + m] = 1j * s2
            U[a, l - m] = -1j * (-1) ** m * s2
    return U


def _real_cg(l1, l2, l3):
    Cc = np.zeros((2 * l1 + 1, 2 * l2 + 1, 2 * l3 + 1), dtype=complex)
    for a, m1 in enumerate(range(-l1, l1 + 1)):
        for b, m2 in enumerate(range(-l2, l2 + 1)):
            for c, m3 in enumerate(range(-l3, l3 + 1)):
                Cc[a, b, c] = _su2_cg(l1, l2, l3, m1, m2, m3)
    U1, U2, U3 = _u_c2r(l1), _u_c2r(l2), _u_c2r(l3)
    C = np.einsum('ap,bq,cr,pqr->abc', np.conj(U1), np.conj(U2), U3, Cc)
    C = C.imag if np.abs(C.imag).max() > np.abs(C.real).max() else C.real
    nrm = np.linalg.norm(C)
    return np.asarray(C / max(nrm, 1e-12), dtype=np.float32)


CG = {lls: _real_cg(*lls) for lls in set(PATHS) | {(1, 1, 2)}}


def _triples():
    """All nonzero CG entries: (path, l1, l2, l3, m, k, c, val)."""
    out = []
    for p, (l1, l2, l3) in enumerate(PATHS):
        C = CG[(l1, l2, l3)]
        for m in range(2 * l1 + 1):
            for k in range(2 * l2 + 1):
                for c in range(2 * l3 + 1):
                    v = float(np.float32(C[m, k, c]))
                    if abs(v) > 1e-8:
                        out.append((p, l1, l2, l3, m, k, c, v))
    return out


TRIPLES = _triples()

# variant table: (path, val) -> (w2s column offset, width)
def _variants():
    offs = {}
    widths = {}
    tot = 0
    for (p, l1, l2, l3, m, k, c, v) in TRIPLES:
        key = (p, v)
        if key not in offs:
            w = 96 if l3 == 0 else 48
            offs[key] = tot
            widths[key] = w
            tot += w
    return offs, widths, tot


W2S_OFF, W2S_W, W2S_TOT = _variants()

_CACHE = {}


def _build_nc(dense: bool, prec: str = "f32r", nw: int = NMAX):
    sys.path.insert(0, '/opt/trn_rl_repo')
    import concourse.bass as bass
    import concourse.mybir as mybir
    import concourse.tile as tile
    from concourse import bacc

    fp32 = mybir.dt.float32
    AOP = mybir.AluOpType
    # adt: linear-chain dtype (wls/tpt/XT/Y); pdt: products + w2s dtype
    adt = {"f32r": mybir.dt.float32r, "mixed": mybir.dt.float16,
           "bf16": mybir.dt.bfloat16, "fp32": mybir.dt.float32}[prec]
    pdt = {"f32r": mybir.dt.float32r, "mixed": mybir.dt.float16,
           "bf16": mybir.dt.bfloat16, "fp32": mybir.dt.float32}[prec]
    nc = bacc.Bacc("TRN2", target_bir_lowering=False)
    # x arrives host-transposed+planar: [48 u, (l,i)-planes * 128 nodes]
    x = nc.dram_tensor("x", [F, 9 * 128], adt, kind="ExternalInput")
    wls = nc.dram_tensor("wls", [F, 3 * F], adt, kind="ExternalInput")
    tpt = nc.dram_tensor("tpt", [F, 11 * F], adt, kind="ExternalInput")
    w2s = nc.dram_tensor("w2s", [F, W2S_TOT], pdt, kind="ExternalInput")
    out = nc.dram_tensor("out", [NMAX, P, D, D], fp32, kind="ExternalOutput")
    out_flat = out.rearrange("n p a b -> n p (a b)")
    dbg = os.environ.get("BASS_KERNEL_DEBUG", "0") == "1"
    if dbg:
        dbg_y = {l: nc.dram_tensor(f"dbg_y{l}", [F, (2 * l + 1) * 128], fp32,
                                   kind="ExternalOutput") for l in (0, 1, 2)}
        dbg_g = {l: nc.dram_tensor(f"dbg_g{l}", [NMAX, (96, 144, 240)[l]], fp32,
                                   kind="ExternalOutput") for l in (0, 1, 2)}
        dbg_prod = nc.dram_tensor("dbg_prod", [F, 25 * 128], fp32,
                                  kind="ExternalOutput")

    C112 = CG[(1, 1, 2)]
    s3 = float(np.float32(3.0 ** -0.5))

    def apv(base, extra_off, dims):
        """Custom strided free-dim view: keep partition dim, replace free dims."""
        ap = base.copy()
        newap = [list(ap.ap[0])] + [list(dd) for dd in dims]
        return ap.__replace__(offset=ap.offset + extra_off, ap=newap)

    with tile.TileContext(nc) as tc:
        with tc.tile_pool(name="const", bufs=1) as cpool, \
             tc.tile_pool(name="work", bufs=1) as wpool, \
             tc.tile_pool(name="xtp", bufs=2) as xtpool, \
             tc.tile_pool(name="pp", bufs=3) as ppool, \
             tc.tile_pool(name="psy", bufs=1, space="PSUM") as psy, \
             tc.tile_pool(name="psa", bufs=2, space="PSUM") as psa, \
             tc.tile_pool(name="psg", bufs=1, space="PSUM") as psg:

            xt = wpool.tile([F, 9 * 128], adt, tag="x")
            nc.sync.dma_start(out=xt[:], in_=x[:])
            wls_sb = cpool.tile([F, 3 * F], adt, tag="wls")
            nc.sync.dma_start(out=wls_sb[:], in_=wls[:])
            tpt_sb = cpool.tile([F, 11 * F], adt, tag="tpt")
            nc.sync.dma_start(out=tpt_sb[:], in_=tpt[:])
            w2s_sb = cpool.tile([F, W2S_TOT], pdt, tag="w2s")
            nc.sync.dma_start(out=w2s_sb[:], in_=w2s[:])

            if dense:
                img = wpool.tile([NMAX, D * D], fp32, tag="img")
                nc.gpsimd.memset(img[:], 0.0)

            # ---- PE warm-up: ~4us of dummy matmuls on the first-loaded
            # weight tile so HAM reaches K=8/8 before the real work ----
            wsrc = cpool.tile([F, 512], mybir.dt.bfloat16, tag="wsrc")
            nc.vector.memset(wsrc[:], 0.0)
            wup = psy.tile([F, 512], fp32, tag="warm", name="warm")
            for _ in range(9):
                nc.tensor.matmul(wup[:], wsrc[:, 0:48], wsrc[:],
                                 start=True, stop=True)

            # ---- y ----
            Y = {}
            PLOFF = {0: 0, 1: 128, 2: 4 * 128}  # plane offsets in xt columns
            for l in (0, 1, 2):
                ni = 2 * l + 1
                Y[l] = wpool.tile([F, ni * 128], adt, tag=f"y{l}", name=f"y{l}")
                for c0 in range(0, ni * 128, 512):
                    c1 = min(c0 + 512, ni * 128)
                    pyy = psy.tile([F, 512], fp32, tag="yps")
                    nc.tensor.matmul(pyy[:, 0:c1 - c0],
                                     wls_sb[:, l * F:(l + 1) * F],
                                     xt[:, PLOFF[l] + c0:PLOFF[l] + c1],
                                     start=True, stop=True)
                    nc.vector.tensor_copy(Y[l][:, c0:c1], pyy[:, 0:c1 - c0])

            if dbg:
                for l in (0, 1, 2):
                    nc.sync.dma_start(out=dbg_y[l][:], in_=Y[l][:])

            # ---- g accumulators in PSUM ----
            G = {0: psg.tile([NMAX, 96], fp32, tag="g0", name="g0"),
                 1: psg.tile([NMAX, 144], fp32, tag="g1", name="g1"),
                 2: psg.tile([NMAX, 240], fp32, tag="g2", name="g2")}

            # start/stop at PSUM-bank granularity: one accumulation group
            # per G tile (start marks the whole 2KB zero-region pending).
            emit_order = [t for p in ([q for q in range(11) if PATHS[q][2] < 2]
                                      + [q for q in range(11) if PATHS[q][2] == 2])
                          for t in TRIPLES if t[0] == p]
            bank_trips = {}
            for t in emit_order:
                bank_trips.setdefault(t[3], []).append(t)
            first_of_bank = {b: ts[0] for b, ts in bank_trips.items()}
            last_of_bank = {b: ts[-1] for b, ts in bank_trips.items()}

            # ---- per-path: a matmul, products, triple matmuls ----
            # l3=2 paths last so the i=0 output rows (needing only g0/g1)
            # can be assembled + scattered while l3=2 still computes
            path_order = [p for p in range(11) if PATHS[p][2] < 2] + \
                         [p for p in range(11) if PATHS[p][2] == 2]
            for p in path_order:
                (l1, l2, l3) = PATHS[p]
                m1n = 2 * l1 + 1
                k2n = 2 * l2 + 1
                # a matmuls, in k-groups that fit one PSUM bank (<=4 k-planes)
                kgs = [(0, min(4, k2n))] + ([(4, k2n)] if k2n > 4 else [])
                prod = ppool.tile([F, m1n, k2n, 128], pdt, tag="prod")
                a16 = xtpool.tile([F, 5 * 128], adt, tag="a16", name="a16")
                for (k0, k1) in kgs:
                    kw = k1 - k0
                    pa = psa.tile([F, 4 * 128], fp32, tag="aps")
                    nc.tensor.matmul(pa[:, 0:kw * 128],
                                     tpt_sb[:, p * F:(p + 1) * F],
                                     Y[l2][:, k0 * 128:k1 * 128],
                                     start=True, stop=True)
                    nc.scalar.copy(a16[:, k0 * 128:k1 * 128], pa[:, 0:kw * 128])
                # products: P[m, k, n] = y1[m, n] * a[k, n]
                y1v = Y[l1][:].rearrange("p (m n) -> p m n", m=m1n)
                y1b = y1v[:, :, None, :].to_broadcast((F, m1n, k2n, 128))
                av = a16[:, 0:k2n * 128].rearrange("p (k n) -> p k n", k=k2n)
                ab = av[:, None, :, :].to_broadcast((F, m1n, k2n, 128))
                nc.vector.tensor_tensor(out=prod[:], in0=y1b, in1=ab,
                                        op=AOP.mult)
                if dbg and p == 10:
                    nc.sync.dma_start(out=dbg_prod[:],
                                      in_=prod[:].rearrange("p a b c -> p (a b c)"))
                # triple matmuls for this path
                for t in TRIPLES:
                    tp, _, _, tl3, m, k, c, v = t
                    if tp != p:
                        continue
                    off = W2S_OFF[(p, v)]
                    w = W2S_W[(p, v)]
                    if tl3 == 0:
                        gdst = G[0][:, 0:96]
                    else:
                        gdst = G[tl3][:, c * F:(c + 1) * F]
                    nc.tensor.matmul(gdst,
                                     prod[:, m, k, :],
                                     w2s_sb[:, off:off + w],
                                     start=(first_of_bank[tl3] is t),
                                     stop=(last_of_bank[tl3] is t),
                                     skip_group_check=True)

            if dbg:
                for l in (0, 1, 2):
                    gs = wpool.tile([NMAX, (96, 144, 240)[l]], fp32, tag=f"dbggs{l}", name=f"dbggs{l}")
                    nc.vector.tensor_copy(gs[:], G[l][:])
                    nc.sync.dma_start(out=dbg_g[l][:], in_=gs[:])

            # ---- M assembly ----
            C112v = CG[(1, 1, 2)]
            s3c = float(np.float32(3.0 ** -0.5))
            a0 = G[0][:, 0:48]
            g1v = apv(G[1][:], 0, [[1, 48], [48, 3]])
            a1b = apv(G[0][:], 48, [[1, 48], [0, 3]])
            a1b1 = apv(G[0][:], 48, [[1, 48]])
            d2b2 = apv(G[2][:], 2 * 48, [[1, 48], [0, 2]])
            d2b1 = apv(G[2][:], 2 * 48, [[1, 48]])
            d4b1 = apv(G[2][:], 4 * 48, [[1, 48]])

            def assemble(dst, fs, istr, jstr, i0dst=None, i0fs=0, i0js=0):
                """Write M values. If i0dst given, row i=0 goes there
                (i0fs/i0js strides), rows i>=1 go to dst with (i-1) indexing
                via istr."""
                dfull = dst[:]

                def dv(off, dims):
                    return apv(dfull, off, dims)

                if i0dst is None:
                    t, tfs, tjs, ioff = dst, fs, jstr, 0
                else:
                    t, tfs, tjs, ioff = i0dst, i0fs, i0js, 0
                tf = t[:]
                nc.vector.tensor_copy(apv(tf, ioff, [[tfs, 48]]), a0)
                nc.vector.tensor_copy(apv(tf, ioff + tjs, [[tfs, 48], [tjs, 3]]),
                                      g1v)
                # base offset of row i (i>=1) inside dst
                def ro(i, j):
                    ii = i - 1 if i0dst is not None else i
                    return ii * istr + j * jstr
                # first column (i,0) = v
                nc.vector.tensor_copy(dv(ro(1, 0), [[fs, 48], [istr, 3]]), g1v)
                # diag a1/sqrt3
                dstep = ro(2, 2) - ro(1, 1)
                nc.vector.tensor_scalar_mul(
                    dv(ro(1, 1), [[fs, 48], [dstep, 3]]), a1b, s3c)
                nc.vector.scalar_tensor_tensor(
                    out=dv(ro(1, 1), [[fs, 48], [2 * dstep, 2]]), in0=d2b2,
                    scalar=float(C112v[0, 0, 2]),
                    in1=dv(ro(1, 1), [[fs, 48], [2 * dstep, 2]]),
                    op0=AOP.mult, op1=AOP.add)
                nc.vector.scalar_tensor_tensor(
                    out=dv(ro(2, 2), [[fs, 48]]), in0=d2b1,
                    scalar=float(C112v[1, 1, 2]),
                    in1=dv(ro(2, 2), [[fs, 48]]), op0=AOP.mult, op1=AOP.add)
                nc.vector.scalar_tensor_tensor(
                    out=dv(ro(1, 1), [[fs, 48]]), in0=d4b1,
                    scalar=float(C112v[0, 0, 4]),
                    in1=dv(ro(1, 1), [[fs, 48]]), op0=AOP.mult, op1=AOP.add)
                nc.vector.scalar_tensor_tensor(
                    out=dv(ro(3, 3), [[fs, 48]]), in0=d4b1,
                    scalar=float(C112v[2, 2, 4]),
                    in1=dv(ro(3, 3), [[fs, 48]]), op0=AOP.mult, op1=AOP.add)
                for (dm, r, cc, coef) in ((1, 1, 2, float(C112v[0, 1, 1])),
                                          (0, 1, 3, float(C112v[0, 2, 0])),
                                          (3, 2, 3, float(C112v[1, 2, 3]))):
                    o1, o2 = ro(r, cc), ro(cc, r)
                    dmb = apv(G[2][:], dm * 48, [[1, 48], [0, 2]])
                    nc.vector.tensor_scalar_mul(
                        dv(min(o1, o2), [[fs, 48], [abs(o2 - o1), 2]]),
                        dmb, coef)

            if dense:
                img_t = img
                assemble(img_t, 772, 192, 1)
                for pp_ in range(P):
                    nc.sync.dma_start(out=out_flat[0:nw, pp_], in_=img[0:nw, :])
            else:
                m_i0 = wpool.tile([NMAX, 4 * 48], fp32, tag="mi0", name="mi0")
                m_rest = wpool.tile([NMAX, 12 * 48], fp32, tag="mrest",
                                    name="mrest")
                assemble(m_rest, 12, 4, 1, i0dst=m_i0, i0fs=4, i0js=1)
                mi0v = m_i0[0:nw, :].rearrange("n (f j) -> n f j", j=4)
                mrv = m_rest[0:nw, :].rearrange("n (f i j) -> n f i j", i=3, j=4)
                def dap_for(pp_, i):
                    base = out_flat[:, pp_]
                    nap = [list(base.ap[0]), [772, 48], [1, 4]]
                    nap[0][1] = nw
                    return base.__replace__(
                        offset=base.offset + 192 * i, ap=nap)
                # i=0 rows go via the two HWDGE rings (sync/scalar) so they
                # stream concurrently with the SWDGE m_rest scatter
                nc.sync.dma_start(out=dap_for(0, 0), in_=mi0v)
                nc.scalar.dma_start(out=dap_for(1, 0), in_=mi0v)
                for pp_ in range(P):
                    for i in range(1, 4):
                        nc.gpsimd.dma_start(out=dap_for(pp_, i),
                                            in_=mrv[:, :, i - 1, :])
    nc.finalize()
    return nc


def _get_nc(dense, prec, nw=NMAX):
    key = (dense, prec, nw)
    if key not in _CACHE:
        _CACHE[key] = _build_nc(dense, prec, nw)
    return _CACHE[key]


def _host_weights(W_lin0, W_lin1, W_lin2, tp_w, W_out0, W_out1, W_out2,
                  Wt0, Wt1, Wt2, prec="f32r"):
    sF = np.float32(F) ** -0.5
    sK = np.float32(K) ** -0.5
    wls = np.concatenate([W_lin0, W_lin1, W_lin2], axis=1) * sF      # [48,144]
    tpt = np.concatenate([tp_w[p].T for p in range(11)], axis=1) * sF
    Wt = {0: Wt0, 1: Wt1, 2: Wt2}
    Wo = {0: W_out0, 1: W_out1, 2: W_out2}
    # W2_l[u', f, o] with h-norm and K-norm folded
    W2 = {}
    for l in (0, 1, 2):
        nrm = np.float32(NP_L[l] * F) ** -0.5
        W2[l] = np.einsum('ukf,ko->ufo',
                          Wo[l].reshape(-1, K, F), Wt[l]) * (nrm * sK)
    w2s = np.zeros((F, W2S_TOT), np.float32)
    for (p, v), off in W2S_OFF.items():
        l3 = PATHS[p][2]
        block = W2[l3][p_slice_start(p) * F:(p_slice_start(p) + 1) * F]
        if l3 == 0:
            # columns o*48+f
            w2s[:, off:off + 96] = (v * block.transpose(0, 2, 1)
                                    .reshape(F, 96))
        else:
            w2s[:, off:off + 48] = v * block[:, :, 0].reshape(F, F).T.T
    import ml_dtypes
    adt = {"f32r": np.float32, "mixed": np.float16,
           "bf16": ml_dtypes.bfloat16, "fp32": np.float32}[prec]
    pdt = {"f32r": np.float32, "mixed": np.float16,
           "bf16": ml_dtypes.bfloat16, "fp32": np.float32}[prec]
    return (np.ascontiguousarray(wls.astype(adt)),
            np.ascontiguousarray(tpt.astype(adt)),
            np.ascontiguousarray(w2s.astype(pdt)))


def p_slice_start(p):
    """index of path p within its l3 group (for W_out row blocks)."""
    l3 = PATHS[p][2]
    return sum(1 for q in range(p) if PATHS[q][2] == l3)


def _shard_nodes(node_feats, batch, prec="mixed"):
    import ml_dtypes
    adt = {"f32r": np.float32, "mixed": np.float16,
           "bf16": ml_dtypes.bfloat16, "fp32": np.float32}[prec]
    nf = np.asarray(node_feats, np.float32)
    bt = np.asarray(batch).astype(np.int64)
    xs = np.zeros((B, NMAX, NFEAT), np.float32)
    cnts = np.zeros(B, np.int32)
    for g in range(B):
        rows = nf[bt == g]
        cnt = min(len(rows), NMAX)
        xs[g, :cnt] = rows[:cnt]
        cnts[g] = cnt
    # transpose to [48 u, (l,i)-planes * 128] planar layout
    xtp = np.zeros((B, F, 9 * 128), np.float32)
    col = 0
    for l in (0, 1, 2):
        ni = 2 * l + 1
        for i in range(ni):
            blk = xs[:, :, LOFF[l] + i:LOFF[l] + ni * F:ni]   # [B, 128, 48]
            xtp[:, :, col * 128:(col + 1) * 128] = blk.transpose(0, 2, 1)
            col += 1
    return np.ascontiguousarray(xtp.astype(adt)), cnts


def _install_ntff_hook():
    """Provide the antenv.axon_hooks module the boot silently skipped, and
    register the ctypes-based NTFF profile hook so trace=True works."""
    try:
        sys.path.insert(0, '/root/.axon_site')
        import antenv
        shim = os.path.join(os.path.dirname(os.path.abspath(__file__)),
                            'work', 'shim_ax')
        if shim not in antenv.__path__ and os.path.isdir(shim):
            antenv.__path__.append(shim)
        import importlib
        try:
            ah = importlib.import_module('antenv.axon_hooks')
        except ImportError:
            return False
        if ah.get_axon_ntff_profile_hook() is None:
            from trn_agent_boot.trn_boot import _ntff_profile_via_ctypes
            ah.set_axon_ntff_profile_hook(
                _ntff_profile_via_ctypes('/opt/axon/libaxon_pjrt.so'))
        return True
    except Exception:
        return False


def _run(inputs, trace=False):
    dense = os.environ.get("BASS_KERNEL_DENSE", "0") == "1"
    prec = os.environ.get("BASS_KERNEL_PREC", "mixed")
    sys.path.insert(0, '/opt/trn_rl_repo')
    if trace:
        _install_ntff_hook()
    from concourse.bass_utils import run_bass_kernel_spmd
    wls, tpt, w2s = _host_weights(
        np.asarray(inputs['W_lin0'], np.float32),
        np.asarray(inputs['W_lin1'], np.float32),
        np.asarray(inputs['W_lin2'], np.float32),
        np.asarray(inputs['tp_w'], np.float32),
        np.asarray(inputs['W_out0'], np.float32),
        np.asarray(inputs['W_out1'], np.float32),
        np.asarray(inputs['W_out2'], np.float32),
        np.asarray(inputs['Wt0'], np.float32),
        np.asarray(inputs['Wt1'], np.float32),
        np.asarray(inputs['Wt2'], np.float32), prec=prec)
    xs, cnts = _shard_nodes(inputs['node_feats'], inputs['batch'], prec)
    # bake the max populated-slot count into the compiled output DMAs;
    # slots beyond it stay runtime-pre-zeroed
    nw = min(NMAX, int(-(-int(cnts.max()) // 8) * 8)) if cnts.max() > 0 else 8
    nc = _get_nc(dense, prec, nw)
    in_maps = [{"x": np.ascontiguousarray(xs[g]), "wls": wls, "tpt": tpt,
                "w2s": w2s} for g in range(B)]
    res = run_bass_kernel_spmd(nc, in_maps, core_ids=list(range(B)),
                               trace=trace)
    stacked = np.stack([np.asarray(res.results[g]["out"]) for g in range(B)])
    max_nodes = int(np.asarray(inputs['max_nodes']))
    if max_nodes == NMAX:
        full = stacked
    elif max_nodes < NMAX:
        full = stacked[:, :max_nodes]
    else:
        full = np.zeros((B, max_nodes, P, D, D), np.float32)
        full[:, :NMAX] = stacked
    return full, res


def kernel(**inputs):
    full, _ = _run(inputs, trace=False)
    return full
